# revision 1
# baseline (speedup 1.0000x reference)
"""Trainium2 kernel for nn_Loss_26886495273741 (retrieval_knn).

reference:
    dots = feature @ feature.T          # [n, n], n=16384, d=256
    dots[diag] = -1
    I = argmax(dots, axis=1)
    loss = -mean(log(n * ||feature - feature[I] + 1e-6||_2))

Strategy (8 NeuronCores, SPMD, no collectives):
  * Rows are sharded: core c owns rows [c*2048, (c+1)*2048).
  * Host passes F^T in bf16 twice: full ("ft", identical on all cores,
    the "all-gather" done by host replication) and the core's own row
    block ("at").
  * Device, per 128-row tile: bf16 matmuls accumulate 32 PSUM chunks of
    [128, 512] fp32 dots; ACT/DVE copy-cast them to a bf16 row
    [128, 32*512]; DVE folds the row in-place with tensor-tensor max
    (bf16 2x mode) down to 512 "position classes" (class = col mod 512);
    max8 gives the top-2 class values (top-1 is always the diagonal
    self-dot, top-2 the best neighbour); max_index gives the class of
    the top-2 value.  Only that class index (u32) is shipped back.
  * Host recovers the exact argmax by evaluating, in fp32, the 32
    candidate columns of the device-reported class plus the 32 columns
    of the diagonal's class (union kills the 1/512 class-collision
    case), then computes the reference loss formula exactly.

The final loss is insensitive to near-tie argmax flips (each row
contributes 1/16384 of a log-term); measured end-to-end relative error
of this pipeline vs the fp32 reference is ~4e-6.
"""

import os
import sys

import numpy as np

# The axon PJRT plugin must be selectable: if a harness pinned
# JAX_PLATFORMS=cpu (common for running jax references), the device run
# would see no NeuronCores.  Prepending axon is a no-op when unset.
_jp = os.environ.get("JAX_PLATFORMS")
if _jp is not None and "axon" not in _jp:
    os.environ["JAX_PLATFORMS"] = "axon," + _jp

try:
    import concourse.bass as bass  # noqa: F401
except ImportError:  # grading env runs from a bare directory
    sys.path.insert(0, "/opt/trn_rl_repo")

import ml_dtypes

import concourse.bass as bass
import concourse.mybir as mybir
import concourse.tile as tile
from concourse import bacc
from concourse.bass_utils import run_bass_kernel_spmd

# Problem geometry (hardcoded per spec.json: feature [16384, 256] f32).
N = 16384
D = 256
N_CORES = 8
ROWS_PER_CORE = N // N_CORES  # 2048
P = 128  # SBUF partitions
ROW_TILES = ROWS_PER_CORE // P  # 16
CHUNK = 512  # matmul free dim == one PSUM bank (fp32)
N_CHUNKS = N // CHUNK  # 32
KH = D // P  # 2 contraction halves
W = 256  # fold width == number of position classes (host recovers N//W cands)

EPS = 1e-6

_BF16 = mybir.dt.bfloat16
_F32 = mybir.dt.float32
_U32 = mybir.dt.uint32
_FP8 = mybir.dt.float8e4
_FP8_NP = mybir.dt.np(_FP8)

USE_FP8 = True  # fp8 DoubleRow matmul (PE 2x); loss rel err ~2.4e-5 vs ~4e-6 bf16


PSUM_BANKS = 2  # banks per psum tile (2 or 4)
PSW = PSUM_BANKS * CHUNK  # psum tile width in f32 elements
N_PS = N // PSW  # psum tiles per row-tile


def build_nc(
    absorb_period: int = 4,
    use_fp8: bool = USE_FP8,
    psum_banks: int = PSUM_BANKS,
    s_bufs: int = 2,
):
    """Build the per-core Bass module.

    Per 128-row tile, 16 two-bank PSUM tiles [128, 1024] are produced by
    the PE.  PSUM tiles with (i % absorb_period == 1) are absorbed by the
    vector engine directly into the fold (TT-max PSUM x SBUF -> SBUF,
    saving their scalar-engine copy); the rest are copy-cast to bf16 by
    the scalar engine.  The surviving bf16 tiles are tree-folded with
    TT-max (bf16 2x mode) down to one [128, 1024] tile, halved to the
    512 position classes, then max8/max_index give the top-2 value and
    its class.
    """
    psw = psum_banks * CHUNK
    n_ps = N // psw
    chunks_per_ps = psw // CHUNK
    psum_bufs = 8 // psum_banks

    nc = bacc.Bacc("TRN2", target_bir_lowering=False, debug=False)

    in_dt = _FP8 if use_fp8 else _BF16
    # layout [P, KH, cols]: partition = k % 128, then k-half, then column
    ft_dram = nc.dram_tensor("ft", [P, KH, N], in_dt, kind="ExternalInput")
    at_dram = nc.dram_tensor("at", [P, KH, ROWS_PER_CORE], in_dt, kind="ExternalInput")
    idx_dram = nc.dram_tensor("idx", [ROWS_PER_CORE, 3], _U32, kind="ExternalOutput")

    absorb = [i % absorb_period == 1 for i in range(n_ps)]
    # absorbed tile i folds onto the copy target of tile i-1
    for i in range(n_ps):
        if absorb[i]:
            assert i > 0 and not absorb[i - 1]

    with tile.TileContext(nc) as tc:
        with (
            tc.tile_pool(name="ft_pool", bufs=1) as ft_pool,
            tc.tile_pool(name="at_pool", bufs=1) as at_pool,
            tc.tile_pool(name="s_pool", bufs=s_bufs) as s_pool,
            tc.tile_pool(name="small_pool", bufs=2) as small_pool,
            tc.tile_pool(name="psum", bufs=psum_bufs, space="PSUM") as psum_pool,
        ):
            # Resident operands: F^T [128, 2, 16384] and the core's own
            # row block A^T [128, 2, 2048] (k-halves on the middle axis).
            at_sb = at_pool.tile([P, KH, ROWS_PER_CORE], in_dt, tag="at")
            nc.sync.dma_start(at_sb[:], at_dram[:])
            ft_sb = ft_pool.tile([P, KH, N], in_dt, tag="ft")
            # split the big load into column blocks, first blocks first so
            # the first row-tile's matmuls and drains can start early
            for j in range(0, N, 1024):
                nc.sync.dma_start(
                    ft_sb[:, :, j : j + 1024], ft_dram[:, :, j : j + 1024]
                )

            for r in range(ROW_TILES):
                s_tiles = []  # bf16 [P, PSW] partial-fold tiles
                target_of = {}
                for i in range(n_ps):
                    ps = psum_pool.tile([P, psw], _F32, tag="ps")
                    for h in range(chunks_per_ps):
                        c = chunks_per_ps * i + h
                        if use_fp8:
                            nc.tensor.matmul(
                                ps[:, h * CHUNK : (h + 1) * CHUNK],
                                at_sb[:, :, r * P : (r + 1) * P],
                                ft_sb[:, :, c * CHUNK : (c + 1) * CHUNK],
                                start=True,
                                stop=True,
                                perf_mode=mybir.MatmulPerfMode.DoubleRow,
                            )
                        else:
                            for kh in range(KH):
                                nc.tensor.matmul(
                                    ps[:, h * CHUNK : (h + 1) * CHUNK],
                                    at_sb[:, kh, r * P : (r + 1) * P],
                                    ft_sb[:, kh, c * CHUNK : (c + 1) * CHUNK],
                                    start=(kh == 0),
                                    stop=(kh == KH - 1),
                                )
                    if absorb[i]:
                        tgt = target_of[i - 1]
                        nc.vector.tensor_tensor(
                            tgt[:], ps[:], tgt[:], mybir.AluOpType.max
                        )
                    else:
                        s = s_pool.tile([P, psw], _BF16, tag=f"s{len(s_tiles)}")
                        nc.scalar.copy(s[:], ps[:])
                        target_of[i] = s
                        s_tiles.append(s)

                # bf16 tree fold of the s tiles down to s_tiles[0]
                live = list(range(len(s_tiles)))
                while len(live) > 1:
                    nxt = []
                    for j in range(0, len(live) - 1, 2):
                        a, b = live[j], live[j + 1]
                        nc.vector.tensor_tensor(
                            s_tiles[a][:],
                            s_tiles[a][:],
                            s_tiles[b][:],
                            mybir.AluOpType.max,
                        )
                        nxt.append(a)
                    if len(live) % 2:
                        nxt.append(live[-1])
                    live = nxt
                root = s_tiles[live[0]]
                # halve down to the W position classes
                width = psw
                while width > W:
                    half = width // 2
                    nc.vector.tensor_tensor(
                        root[:, 0:half],
                        root[:, 0:half],
                        root[:, half:width],
                        mybir.AluOpType.max,
                    )
                    width = half

                f = root[:, 0:W]  # [P, W] class maxima
                g8 = small_pool.tile([P, 8], _BF16, tag="g8")
                ti = small_pool.tile([P, 8], _U32, tag="ti")
                nc.vector.max(out=g8[:], in_=f)
                nc.vector.max_index(out=ti[:], in_max=g8[:], in_values=f)

                # ship the classes of the top-2..4 fold values (top-1 is the
                # self-dot); host unions them with the diagonal class
                nc.sync.dma_start(idx_dram[r * P : (r + 1) * P, :], ti[:, 1:4])

    nc.compile()
    return nc


_NC_CACHE = {}


def _get_nc():
    if "nc" not in _NC_CACHE:
        _NC_CACHE["nc"] = build_nc()
    return _NC_CACHE["nc"]


def make_inputs(feature: np.ndarray):
    """Host-side shard prep: F^T in [P, KH, cols] layout, quantized."""
    np_dt = _FP8_NP if USE_FP8 else ml_dtypes.bfloat16
    # ft[p, kh, j] = feature[j, kh*P + p]
    ft = np.ascontiguousarray(
        feature.T.reshape(KH, P, N).transpose(1, 0, 2)
    ).astype(np_dt)
    in_maps = []
    for c in range(N_CORES):
        at = np.ascontiguousarray(
            ft[:, :, c * ROWS_PER_CORE : (c + 1) * ROWS_PER_CORE]
        )
        in_maps.append({"ft": ft, "at": at})
    return in_maps


def run_device(feature: np.ndarray, trace: bool = False):
    """Run the SPMD kernel; returns (t_cls [N] u32, results obj)."""
    nc = _get_nc()
    in_maps = make_inputs(feature)
    res = run_bass_kernel_spmd(nc, in_maps, core_ids=list(range(N_CORES)), trace=trace)
    t_cls = np.concatenate([r["idx"] for r in res.results]).astype(np.int64)  # [N, 3]
    return t_cls, res


def recover_loss(feature: np.ndarray, t_cls: np.ndarray) -> np.float32:
    """Exact argmax recovery + reference loss formula on host.

    For each row the true argmax column is recovered from the union of
    two "position classes" (columns congruent mod W): the device-reported
    class of the bf16 max, and the row's own diagonal class (covers the
    rare case where the best neighbour hides under the self-dot in the
    fold).  Rows are processed grouped by class so candidate dot products
    are real GEMMs.
    """
    n = feature.shape[0]
    B = n // W  # candidate columns per class
    feat = np.ascontiguousarray(feature, dtype=np.float32)
    rows = np.arange(n)
    t_cls = np.atleast_2d(t_cls.T).T  # [n, k] device-reported classes
    # max_index can emit sentinel -1 (0xFFFFFFFF) for duplicate top-8
    # values; replace invalid entries with the row's diagonal class.
    t_cls = np.where((t_cls >= 0) & (t_cls < W), t_cls, (rows % W)[:, None])

    best_val = np.full(n, -np.inf, dtype=np.float32)
    best_col = np.zeros(n, dtype=np.int64)

    def consider(row_idx: np.ndarray, t: int):
        """Evaluate class-t candidate columns for the given rows."""
        cols = t + W * np.arange(B)  # [B]
        cd = feat[row_idx] @ feat[cols].T  # [len(rows), B] exact fp32
        self_b = np.where(row_idx % W == t, row_idx // W, -1)
        k = np.arange(len(row_idx))
        has_self = self_b >= 0
        cd[k[has_self], self_b[has_self]] = -np.inf
        b = np.argmax(cd, axis=1)
        v = cd[k, b]
        c = cols[b]
        upd = (v > best_val[row_idx]) | (
            (v == best_val[row_idx]) & (c < best_col[row_idx])
        )
        ri = row_idx[upd]
        best_val[ri] = v[upd]
        best_col[ri] = c[upd]

    for k in range(t_cls.shape[1]):
        col = t_cls[:, k]
        order = np.argsort(col, kind="stable")
        bounds = np.searchsorted(col[order], np.arange(W + 1))
        for t in range(W):
            grp = order[bounds[t] : bounds[t + 1]]
            if len(grp):
                consider(grp, t)
    for t in range(W):
        consider(rows[t::W], t)  # rows whose diagonal falls in class t

    I = best_col
    diff = feat - feat[I] + EPS
    dist = np.sqrt((diff * diff).sum(axis=1))
    loss = -np.mean(np.log(n * dist))
    return np.float32(loss)


def kernel(feature: np.ndarray) -> np.ndarray:
    feature = np.asarray(feature, dtype=np.float32)
    try:
        t_cls, _res = run_device(feature)
    except Exception:
        # one retry for transient device/tunnel hiccups
        _NC_CACHE.clear()
        t_cls, _res = run_device(feature)
    return np.asarray(recover_loss(feature, t_cls), dtype=np.float32)


if __name__ == "__main__":
    rng = np.random.default_rng(0)
    feature = rng.standard_normal((N, D), dtype=np.float32)
    print("loss:", kernel(feature))



# revision 6
# speedup vs baseline: 1.3458x; 1.3458x over previous
"""Trainium2 kernel for nn_Loss_26886495273741 (retrieval_knn).

reference:
    dots = feature @ feature.T          # [n, n], n=16384, d=256
    dots[diag] = -1
    I = argmax(dots, axis=1)
    loss = -mean(log(n * ||feature - feature[I] + 1e-6||_2))

Strategy (8 NeuronCores, SPMD, no collectives):
  * Rows are sharded: core c owns rows [c*2048, (c+1)*2048).
  * Host passes F^T in fp8 twice: full ("ft", identical on all cores,
    the "all-gather" done by host replication) and the core's own row
    block ("at").
  * Device, per 128-row tile: fp8 DoubleRow matmuls fill 16 two-bank
    PSUM units [128, 1024] of fp32 dots (covering the 16384 columns).
    PSUM can only be read by the ACT and DVE engines (~1 col/cycle
    each), so the drain is the bottleneck; the 16 units are split
    ~evenly between the two engines (parity-alternating 7/9 and 8/8 so
    the average balances their speeds):
      - DVE max-absorbs its units into two independent bf16
        accumulators (two chains so the per-op semaphore round-trip of
        a single read-modify-write chain is hidden),
      - ACT copy-casts its units to bf16 staging tiles which are
        DMA-shipped to the host raw (the DMA engines are far from
        saturated, and folding them on-device would cost drain
        throughput).
    PSUM cycles as 4 two-bank buffers so up to 4 drains are in flight,
    and unit roles alternate engines so the buffer ring never
    serializes on one engine.  No on-device argmax at all.
  * Host maxes the shipped tiles (position-class maxima, class =
    col mod 1024), picks the top-7 classes per row (plus the
    diagonal's class), and evaluates the 16 candidate columns of each
    selected class in exact fp32 to recover the true argmax, then
    computes the reference loss formula.

The final loss is insensitive to near-tie argmax flips (each row
contributes 1/16384 of a log-term).
"""

import os
import sys

import numpy as np

# The axon PJRT plugin must be selectable: if a harness pinned
# JAX_PLATFORMS=cpu (common for running jax references), the device run
# would see no NeuronCores.  Prepending axon is a no-op when unset.
_jp = os.environ.get("JAX_PLATFORMS")
if _jp is not None and "axon" not in _jp:
    os.environ["JAX_PLATFORMS"] = "axon," + _jp

try:
    import concourse.bass as bass  # noqa: F401
except ImportError:  # grading env runs from a bare directory
    sys.path.insert(0, "/opt/trn_rl_repo")

import concourse.bass as bass
import concourse.mybir as mybir
import concourse.tile as tile
from concourse import bacc
from concourse.bass_utils import run_bass_kernel_spmd

# Problem geometry (hardcoded per spec.json: feature [16384, 256] f32).
N = 16384
D = 256
N_CORES = 8
ROWS_PER_CORE = N // N_CORES  # 2048
P = 128  # SBUF partitions
ROW_TILES = ROWS_PER_CORE // P  # 16
KH = D // P  # 2 contraction halves

UNIT = 1024  # drain unit width == 2 PSUM banks == matmul free dim
N_UNITS = N // UNIT  # 16 units per 128-row tile
W = UNIT  # position classes; host recovers N//W candidate cols per class
MM_WIDTH = 512  # matmul free dim (ISA max 512 per PSUM bank)

N_ACC = 2  # independent DVE accumulator chains

# Per-unit drain engine per row-tile parity: D = DVE, A = ACT.
# Even rows 7 D / 9 A, odd rows 8 D / 8 A (ACT is 1.25x faster per col).
PAT_EVEN = "DAADADAADADAADAD"
PAT_ODD = "DAADADADADADADAD"
NV_EVEN = PAT_EVEN.count("A")  # 9
NV_ODD = PAT_ODD.count("A")  # 8
NV = max(NV_EVEN, NV_ODD)

TOPK = 7  # classes the host refines per row (plus the diagonal class)

EPS = 1e-6

_BF16 = mybir.dt.bfloat16
_F32 = mybir.dt.float32
_FP8 = mybir.dt.float8e4
_FP8_NP = mybir.dt.np(_FP8)

SHIP = tuple(f"md{i}" for i in range(N_ACC)) + tuple(f"mv{i}" for i in range(NV))


def build_nc(mm_width=MM_WIDTH):
    nc = bacc.Bacc("TRN2", target_bir_lowering=False, debug=False)

    # layout [P, KH, cols]: partition = k % 128, then k-half, then column
    ft_dram = nc.dram_tensor("ft", [P, KH, N], _FP8, kind="ExternalInput")
    at_dram = nc.dram_tensor("at", [P, KH, ROWS_PER_CORE], _FP8, kind="ExternalInput")
    outs = {
        name: nc.dram_tensor(name, [ROWS_PER_CORE, W], _BF16, kind="ExternalOutput")
        for name in SHIP
    }

    with tile.TileContext(nc) as tc:
        with (
            tc.tile_pool(name="ft_pool", bufs=1) as ft_pool,
            tc.tile_pool(name="at_pool", bufs=1) as at_pool,
            tc.tile_pool(name="acc_pool", bufs=3) as acc_pool,
            tc.tile_pool(name="s_pool", bufs=3) as s_pool,
            tc.tile_pool(name="psum", bufs=4, space="PSUM") as psum_pool,
        ):
            # Resident operands: F^T [128, 2, 16384] and the core's own
            # row block A^T [128, 2, 2048] (k-halves on the middle axis).
            at_sb = at_pool.tile([P, KH, ROWS_PER_CORE], _FP8, tag="at")
            ft_sb = ft_pool.tile([P, KH, N], _FP8, tag="ft")
            # load order: just what row-tile 0 unit 0 needs first, so the
            # compute pipeline fills as early as possible
            nc.sync.dma_start(at_sb[:, :, 0:128], at_dram[:, :, 0:128])
            nc.sync.dma_start(ft_sb[:, :, 0:1024], ft_dram[:, :, 0:1024])
            nc.sync.dma_start(at_sb[:, :, 128:], at_dram[:, :, 128:])
            for j in range(1024, N, 1024):
                nc.sync.dma_start(
                    ft_sb[:, :, j : j + 1024], ft_dram[:, :, j : j + 1024]
                )

            for r in range(ROW_TILES):
                pat = PAT_EVEN if r % 2 == 0 else PAT_ODD
                accs = [
                    acc_pool.tile([P, W], _BF16, tag=f"accD{i}", name=f"accD{i}_{r}")
                    for i in range(N_ACC)
                ]
                seeded = [False] * N_ACC
                vtiles = {}
                di = 0
                vi = 0
                for u, role in enumerate(pat):
                    ps = psum_pool.tile([P, UNIT], _F32, tag="ps")
                    for k in range(UNIT // mm_width):
                        c0 = u * UNIT + k * mm_width
                        nc.tensor.matmul(
                            ps[:, k * mm_width : (k + 1) * mm_width],
                            at_sb[:, :, r * P : (r + 1) * P],
                            ft_sb[:, :, c0 : c0 + mm_width],
                            start=True,
                            stop=True,
                            perf_mode=mybir.MatmulPerfMode.DoubleRow,
                        )
                    if role == "D":
                        a = accs[di % N_ACC]
                        if not seeded[di % N_ACC]:
                            nc.vector.tensor_copy(a[:], ps[:])
                            seeded[di % N_ACC] = True
                        else:
                            nc.vector.tensor_tensor(
                                a[:], ps[:], a[:], mybir.AluOpType.max
                            )
                        di += 1
                    else:
                        s = s_pool.tile([P, UNIT], _BF16, tag=f"v{vi}", name=f"v{vi}_{r}")
                        nc.scalar.copy(s[:], ps[:])
                        vtiles[f"v{vi}"] = s
                        vi += 1

                for i in range(N_ACC):
                    nc.sync.dma_start(outs[f"md{i}"][r * P : (r + 1) * P, :], accs[i][:])
                for vn, s in vtiles.items():
                    nc.sync.dma_start(outs["m" + vn][r * P : (r + 1) * P, :], s[:])

    nc.compile()
    return nc


_NC_CACHE = {}


def _get_nc():
    if "nc" not in _NC_CACHE:
        _NC_CACHE["nc"] = build_nc()
    return _NC_CACHE["nc"]


def make_inputs(feature: np.ndarray):
    """Host-side shard prep: F^T in [P, KH, cols] layout, quantized."""
    # ft[p, kh, j] = feature[j, kh*P + p]
    ft = np.ascontiguousarray(
        feature.T.reshape(KH, P, N).transpose(1, 0, 2)
    ).astype(_FP8_NP)
    in_maps = []
    for c in range(N_CORES):
        at = np.ascontiguousarray(
            ft[:, :, c * ROWS_PER_CORE : (c + 1) * ROWS_PER_CORE]
        )
        in_maps.append({"ft": ft, "at": at})
    return in_maps


def run_device(feature: np.ndarray, trace: bool = False):
    """Run the SPMD kernel; returns (vals [N, W] f32 class maxima, res)."""
    nc = _get_nc()
    in_maps = make_inputs(feature)
    res = run_bass_kernel_spmd(nc, in_maps, core_ids=list(range(N_CORES)), trace=trace)
    # mv tensors written only by even row-tiles (index >= NV_ODD) must be
    # masked for odd row-tiles (their DRAM is uninitialized there)
    row_tile_even = (np.arange(ROWS_PER_CORE) // P) % 2 == 0
    per_core = []
    for r in res.results:
        vals = r["md0"].astype(np.float32)
        for i in range(1, N_ACC):
            vals = np.maximum(vals, r[f"md{i}"].astype(np.float32))
        for i in range(NV):
            mv = r[f"mv{i}"].astype(np.float32)
            if i >= NV_ODD:
                mv = np.where(row_tile_even[:, None], mv, -np.inf)
            vals = np.maximum(vals, mv)
        per_core.append(vals)
    return np.concatenate(per_core), res


def recover_loss(feature: np.ndarray, vals: np.ndarray) -> np.float32:
    """Exact argmax recovery + reference loss formula on host.

    ``vals[i, c]`` is the device's (fp8-matmul, bf16-cast) max of
    ``dots[i, j]`` over columns j = c (mod W).  The top TOPK classes per
    row (plus the row's own diagonal class, which covers the case where
    the best neighbour hides under the self-dot) are evaluated in exact
    fp32.  Rows are processed grouped by class so candidate dot products
    are real GEMMs.
    """
    n = feature.shape[0]
    B = n // W  # candidate columns per class
    feat = np.ascontiguousarray(feature, dtype=np.float32)
    rows = np.arange(n)
    # top-TOPK classes per row by device value
    t_cls = np.argpartition(-vals, TOPK, axis=1)[:, :TOPK].astype(np.int64)

    best_val = np.full(n, -np.inf, dtype=np.float32)
    best_col = np.zeros(n, dtype=np.int64)

    def consider(row_idx: np.ndarray, t: int):
        """Evaluate class-t candidate columns for the given rows."""
        cols = t + W * np.arange(B)  # [B]
        cd = feat[row_idx] @ feat[cols].T  # [len(rows), B] exact fp32
        self_b = np.where(row_idx % W == t, row_idx // W, -1)
        k = np.arange(len(row_idx))
        has_self = self_b >= 0
        cd[k[has_self], self_b[has_self]] = -np.inf
        b = np.argmax(cd, axis=1)
        v = cd[k, b]
        c = cols[b]
        upd = (v > best_val[row_idx]) | (
            (v == best_val[row_idx]) & (c < best_col[row_idx])
        )
        ri = row_idx[upd]
        best_val[ri] = v[upd]
        best_col[ri] = c[upd]

    for k in range(t_cls.shape[1]):
        col = t_cls[:, k]
        order = np.argsort(col, kind="stable")
        bounds = np.searchsorted(col[order], np.arange(W + 1))
        for t in range(W):
            grp = order[bounds[t] : bounds[t + 1]]
            if len(grp):
                consider(grp, t)
    for t in range(W):
        consider(rows[t::W], t)  # rows whose diagonal falls in class t

    I = best_col
    diff = feat - feat[I] + EPS
    dist = np.sqrt((diff * diff).sum(axis=1))
    loss = -np.mean(np.log(n * dist))
    return np.float32(loss)


def kernel(feature: np.ndarray) -> np.ndarray:
    feature = np.asarray(feature, dtype=np.float32)
    try:
        vals, _res = run_device(feature)
    except Exception:
        # one retry for transient device/tunnel hiccups
        _NC_CACHE.clear()
        vals, _res = run_device(feature)
    return np.asarray(recover_loss(feature, vals), dtype=np.float32)


if __name__ == "__main__":
    rng = np.random.default_rng(0)
    feature = rng.standard_normal((N, D), dtype=np.float32)
    print("loss:", kernel(feature))


# revision 7
# speedup vs baseline: 1.3669x; 1.0156x over previous
"""Trainium2 kernel for nn_Loss_26886495273741 (retrieval_knn).

reference:
    dots = feature @ feature.T          # [n, n], n=16384, d=256
    dots[diag] = -1
    I = argmax(dots, axis=1)
    loss = -mean(log(n * ||feature - feature[I] + 1e-6||_2))

Strategy (8 NeuronCores, SPMD, no collectives):
  * Rows are sharded: core c owns rows [c*2048, (c+1)*2048).
  * Host passes F^T in fp8 twice: full ("ft", identical on all cores,
    the "all-gather" done by host replication) and the core's own row
    block ("at").
  * Device, per 128-row tile: fp8 DoubleRow matmuls fill 16 two-bank
    PSUM units [128, 1024] of fp32 dots (covering the 16384 columns).
    PSUM can only be read by the ACT and DVE engines (~1 col/cycle
    each), so the drain is the bottleneck; the 16 units are split
    ~evenly between the two engines (parity-alternating 7/9 and 8/8 so
    the average balances their speeds):
      - DVE max-absorbs its units into two independent bf16
        accumulators (two chains so the per-op semaphore round-trip of
        a single read-modify-write chain is hidden),
      - ACT copy-casts its units to bf16 staging tiles which are
        DMA-shipped to the host raw (the DMA engines are far from
        saturated, and folding them on-device would cost drain
        throughput).
    PSUM cycles as 4 two-bank buffers so up to 4 drains are in flight,
    and unit roles alternate engines so the buffer ring never
    serializes on one engine.  No on-device argmax at all.
  * Host maxes the shipped tiles (position-class maxima, class =
    col mod 1024), picks the top-7 classes per row (plus the
    diagonal's class), and evaluates the 16 candidate columns of each
    selected class in exact fp32 to recover the true argmax, then
    computes the reference loss formula.

The final loss is insensitive to near-tie argmax flips (each row
contributes 1/16384 of a log-term).
"""

import os
import sys

import numpy as np

# The axon PJRT plugin must be selectable: if a harness pinned
# JAX_PLATFORMS=cpu (common for running jax references), the device run
# would see no NeuronCores.  Prepending axon is a no-op when unset.
_jp = os.environ.get("JAX_PLATFORMS")
if _jp is not None and "axon" not in _jp:
    os.environ["JAX_PLATFORMS"] = "axon," + _jp

try:
    import concourse.bass as bass  # noqa: F401
except ImportError:  # grading env runs from a bare directory
    sys.path.insert(0, "/opt/trn_rl_repo")

import concourse.bass as bass
import concourse.mybir as mybir
import concourse.tile as tile
from concourse import bacc
from concourse.bass_utils import run_bass_kernel_spmd

# Problem geometry (hardcoded per spec.json: feature [16384, 256] f32).
N = 16384
D = 256
N_CORES = 8
ROWS_PER_CORE = N // N_CORES  # 2048
P = 128  # SBUF partitions
ROW_TILES = ROWS_PER_CORE // P  # 16
KH = D // P  # 2 contraction halves

UNIT = 1024  # drain unit width == 2 PSUM banks == matmul free dim
N_UNITS = N // UNIT  # 16 units per 128-row tile
W = UNIT  # position classes; host recovers N//W candidate cols per class
MM_WIDTH = 512  # matmul free dim (ISA max 512 per PSUM bank)

N_ACC = 2  # independent DVE accumulator chains

# Per-unit drain engine per row-tile parity: D = DVE, A = ACT.
# Even rows 7 D / 9 A, odd rows 8 D / 8 A (ACT is 1.25x faster per col).
PAT_EVEN = "ADADAADADADADADA"
PAT_ODD = "ADADADADADADADAD"
NV_EVEN = PAT_EVEN.count("A")  # 9
NV_ODD = PAT_ODD.count("A")  # 8
NV = max(NV_EVEN, NV_ODD)

TOPK = 7  # classes the host refines per row (plus the diagonal class)

EPS = 1e-6

_BF16 = mybir.dt.bfloat16
_F32 = mybir.dt.float32
_FP8 = mybir.dt.float8e4
_FP8_NP = mybir.dt.np(_FP8)

SHIP = tuple(f"md{i}" for i in range(N_ACC)) + tuple(f"mv{i}" for i in range(NV))


def build_nc(mm_width=MM_WIDTH):
    nc = bacc.Bacc("TRN2", target_bir_lowering=False, debug=False)

    # layout [P, KH, cols]: partition = k % 128, then k-half, then column
    ft_dram = nc.dram_tensor("ft", [P, KH, N], _FP8, kind="ExternalInput")
    at_dram = nc.dram_tensor("at", [P, KH, ROWS_PER_CORE], _FP8, kind="ExternalInput")
    outs = {
        name: nc.dram_tensor(name, [ROWS_PER_CORE, W], _BF16, kind="ExternalOutput")
        for name in SHIP
    }

    with tile.TileContext(nc) as tc:
        with (
            tc.tile_pool(name="ft_pool", bufs=1) as ft_pool,
            tc.tile_pool(name="at_pool", bufs=1) as at_pool,
            tc.tile_pool(name="acc_pool", bufs=3) as acc_pool,
            tc.tile_pool(name="s_pool", bufs=3) as s_pool,
            tc.tile_pool(name="psum", bufs=4, space="PSUM") as psum_pool,
        ):
            # Resident operands: F^T [128, 2, 16384] and the core's own
            # row block A^T [128, 2, 2048] (k-halves on the middle axis).
            at_sb = at_pool.tile([P, KH, ROWS_PER_CORE], _FP8, tag="at")
            ft_sb = ft_pool.tile([P, KH, N], _FP8, tag="ft")
            # load order: just what row-tile 0 unit 0 needs first, so the
            # compute pipeline fills as early as possible
            nc.sync.dma_start(at_sb[:, :, 0:128], at_dram[:, :, 0:128])
            nc.sync.dma_start(ft_sb[:, :, 0:1024], ft_dram[:, :, 0:1024])
            nc.sync.dma_start(at_sb[:, :, 128:], at_dram[:, :, 128:])
            for j in range(1024, N, 1024):
                nc.sync.dma_start(
                    ft_sb[:, :, j : j + 1024], ft_dram[:, :, j : j + 1024]
                )

            for r in range(ROW_TILES):
                pat = PAT_EVEN if r % 2 == 0 else PAT_ODD
                accs = [
                    acc_pool.tile([P, W], _BF16, tag=f"accD{i}", name=f"accD{i}_{r}")
                    for i in range(N_ACC)
                ]
                seeded = [False] * N_ACC
                vtiles = {}
                di = 0
                vi = 0
                for u, role in enumerate(pat):
                    ps = psum_pool.tile([P, UNIT], _F32, tag="ps")
                    for k in range(UNIT // mm_width):
                        c0 = u * UNIT + k * mm_width
                        nc.tensor.matmul(
                            ps[:, k * mm_width : (k + 1) * mm_width],
                            at_sb[:, :, r * P : (r + 1) * P],
                            ft_sb[:, :, c0 : c0 + mm_width],
                            start=True,
                            stop=True,
                            perf_mode=mybir.MatmulPerfMode.DoubleRow,
                        )
                    if role == "D":
                        a = accs[di % N_ACC]
                        if not seeded[di % N_ACC]:
                            nc.vector.tensor_copy(a[:], ps[:])
                            seeded[di % N_ACC] = True
                        else:
                            nc.vector.tensor_tensor(
                                a[:], ps[:], a[:], mybir.AluOpType.max
                            )
                        di += 1
                    else:
                        s = s_pool.tile([P, UNIT], _BF16, tag=f"v{vi}", name=f"v{vi}_{r}")
                        nc.scalar.copy(s[:], ps[:])
                        vtiles[f"v{vi}"] = s
                        vi += 1

                for i in range(N_ACC):
                    nc.sync.dma_start(outs[f"md{i}"][r * P : (r + 1) * P, :], accs[i][:])
                for vn, s in vtiles.items():
                    nc.sync.dma_start(outs["m" + vn][r * P : (r + 1) * P, :], s[:])

    nc.compile()
    return nc


_NC_CACHE = {}


def _get_nc():
    if "nc" not in _NC_CACHE:
        _NC_CACHE["nc"] = build_nc()
    return _NC_CACHE["nc"]


def make_inputs(feature: np.ndarray):
    """Host-side shard prep: F^T in [P, KH, cols] layout, quantized."""
    # ft[p, kh, j] = feature[j, kh*P + p]
    ft = np.ascontiguousarray(
        feature.T.reshape(KH, P, N).transpose(1, 0, 2)
    ).astype(_FP8_NP)
    in_maps = []
    for c in range(N_CORES):
        at = np.ascontiguousarray(
            ft[:, :, c * ROWS_PER_CORE : (c + 1) * ROWS_PER_CORE]
        )
        in_maps.append({"ft": ft, "at": at})
    return in_maps


def run_device(feature: np.ndarray, trace: bool = False):
    """Run the SPMD kernel; returns (vals [N, W] f32 class maxima, res)."""
    nc = _get_nc()
    in_maps = make_inputs(feature)
    res = run_bass_kernel_spmd(nc, in_maps, core_ids=list(range(N_CORES)), trace=trace)
    # mv tensors written only by even row-tiles (index >= NV_ODD) must be
    # masked for odd row-tiles (their DRAM is uninitialized there)
    row_tile_even = (np.arange(ROWS_PER_CORE) // P) % 2 == 0
    per_core = []
    for r in res.results:
        vals = r["md0"].astype(np.float32)
        for i in range(1, N_ACC):
            vals = np.maximum(vals, r[f"md{i}"].astype(np.float32))
        for i in range(NV):
            mv = r[f"mv{i}"].astype(np.float32)
            if i >= NV_ODD:
                mv = np.where(row_tile_even[:, None], mv, -np.inf)
            vals = np.maximum(vals, mv)
        per_core.append(vals)
    return np.concatenate(per_core), res


def recover_loss(feature: np.ndarray, vals: np.ndarray) -> np.float32:
    """Exact argmax recovery + reference loss formula on host.

    ``vals[i, c]`` is the device's (fp8-matmul, bf16-cast) max of
    ``dots[i, j]`` over columns j = c (mod W).  The top TOPK classes per
    row (plus the row's own diagonal class, which covers the case where
    the best neighbour hides under the self-dot) are evaluated in exact
    fp32.  Rows are processed grouped by class so candidate dot products
    are real GEMMs.
    """
    n = feature.shape[0]
    B = n // W  # candidate columns per class
    feat = np.ascontiguousarray(feature, dtype=np.float32)
    rows = np.arange(n)
    # top-TOPK classes per row by device value
    t_cls = np.argpartition(-vals, TOPK, axis=1)[:, :TOPK].astype(np.int64)

    best_val = np.full(n, -np.inf, dtype=np.float32)
    best_col = np.zeros(n, dtype=np.int64)

    def consider(row_idx: np.ndarray, t: int):
        """Evaluate class-t candidate columns for the given rows."""
        cols = t + W * np.arange(B)  # [B]
        cd = feat[row_idx] @ feat[cols].T  # [len(rows), B] exact fp32
        self_b = np.where(row_idx % W == t, row_idx // W, -1)
        k = np.arange(len(row_idx))
        has_self = self_b >= 0
        cd[k[has_self], self_b[has_self]] = -np.inf
        b = np.argmax(cd, axis=1)
        v = cd[k, b]
        c = cols[b]
        upd = (v > best_val[row_idx]) | (
            (v == best_val[row_idx]) & (c < best_col[row_idx])
        )
        ri = row_idx[upd]
        best_val[ri] = v[upd]
        best_col[ri] = c[upd]

    for k in range(t_cls.shape[1]):
        col = t_cls[:, k]
        order = np.argsort(col, kind="stable")
        bounds = np.searchsorted(col[order], np.arange(W + 1))
        for t in range(W):
            grp = order[bounds[t] : bounds[t + 1]]
            if len(grp):
                consider(grp, t)
    for t in range(W):
        consider(rows[t::W], t)  # rows whose diagonal falls in class t

    I = best_col
    diff = feat - feat[I] + EPS
    dist = np.sqrt((diff * diff).sum(axis=1))
    loss = -np.mean(np.log(n * dist))
    return np.float32(loss)


def kernel(feature: np.ndarray) -> np.ndarray:
    feature = np.asarray(feature, dtype=np.float32)
    try:
        vals, _res = run_device(feature)
    except Exception:
        # one retry for transient device/tunnel hiccups
        _NC_CACHE.clear()
        vals, _res = run_device(feature)
    return np.asarray(recover_loss(feature, vals), dtype=np.float32)


if __name__ == "__main__":
    rng = np.random.default_rng(0)
    feature = rng.standard_normal((N, D), dtype=np.float32)
    print("loss:", kernel(feature))
